# revision 26
# baseline (speedup 1.0000x reference)
"""Trainium2 Bass kernel for nn_CustomLossTorch_83820581748769.

Block-local projection loss:
  loss = mean_{b,v} || V_bv - P_b (P^H P)^{-1} P^H V_bv ||  over 16x16 spatial blocks.

Strategy (pure data parallel over B=8, one sample per NeuronCore):
  * Host repacks each core's sample into UU[c=4][p=128][g=64][ri=2][96]:
    c = d-chunk (s, t-half), p = (t%8)*16 + x, g = group of 4 blocks,
    96 = 4 blocks x 24 cols (12 pred | 12 target).
  * PE: per group, 9 fp32 matmuls accumulate [A2 | RE | A] (96x288 PSUM):
      RE = Ur^T Ur + Ui^T Ui,  A = Ur^T Ui,  A2 = Ui^T Ur   (4-block diag packing)
  * ACT copies PSUM -> SBUF staging; SBUF->SBUF DMAs extract per-block diag
    24x24 (RE) and 12x24 (A, A2) into a blocks-in-partitions solve layout.
  * DVE: fused complex LDL^H solve batched over 256 blocks (2 slots/partition):
      X = [Re G | Re r], Y = [Im G | Im r], per pivot k:
        invd = 1/X[k,k]; ax = X[k,:]*invd; bx = Y[k,:]*invd
        X[i,c] -= Xc*ax - Yc*bx ;  Y[i,c] -= Yc*ax + Xc*bx   (i,c > k)
      sigma_v = sum_k (zr^2 + zi^2) * invd_k,  con_v = diag(W) - sigma_v
  * ones-matmul reduces con over partitions; sqrt; sum; /96. Host sums 8 scalars.
"""

import sys
import numpy as np

sys.path.insert(0, "/opt/trn_rl_repo")

N_CORES = 8
NV = 12
S = 2
NT = NX = 256
BLK = 16          # spatial block edge
NBT = NBX = 16    # blocks per axis
NBLOCK = NBT * NBX        # 256 blocks per sample
GRP = 4                   # blocks per PE group
NGRP = NBLOCK // GRP      # 64 groups
NCH = 4                   # d-chunks of 128
COLS = 24                 # 12 pred + 12 target cols per block
GCOL = GRP * COLS         # 96
HALF_G = NGRP // 2        # 32 groups per extraction half

_PROGRAM_CACHE = {}


def _host_layout(pred_re, pred_im, target_re, target_im):
    """Full (8,12,2,256,256) f32 inputs -> list of per-core UU arrays."""
    import ml_dtypes
    B = pred_re.shape[0]
    uus = []
    for b in range(B):
        uu = np.empty((NCH, 128, NGRP, 2, GCOL), ml_dtypes.bfloat16)
        # view axes: (s, th, t8, x, bT, bXg, ri, blk, col)
        uuv = uu.reshape(2, 2, 8, 16, 16, 4, 2, 4, 24)
        for ri, (p, t) in enumerate(((pred_re, target_re), (pred_im, target_im))):
            # (n, s, bT, th, t8, bXg, blk, x) -> (s, th, t8, x, bT, bXg, blk, n)
            pv = np.ascontiguousarray(
                p[b].reshape(NV, 2, 16, 2, 8, 4, 4, 16).transpose(1, 3, 4, 7, 2, 5, 6, 0)
            )
            tv = np.ascontiguousarray(
                t[b].reshape(NV, 2, 16, 2, 8, 4, 4, 16).transpose(1, 3, 4, 7, 2, 5, 6, 0)
            )
            uuv[:, :, :, :, :, :, ri, :, 0:12] = pv
            uuv[:, :, :, :, :, :, ri, :, 12:24] = tv
        uus.append(uu)
    return uus


def build_program(debug_dump=False):
    """Build the single-core Bass program (same NEFF for all 8 cores)."""
    key = ("prog", debug_dump)
    if key in _PROGRAM_CACHE:
        return _PROGRAM_CACHE[key]

    import concourse.bass as bass
    import concourse.tile as tile
    from concourse import bacc, mybir

    f32 = mybir.dt.float32
    Alu = mybir.AluOpType
    Act = mybir.ActivationFunctionType

    nc = bacc.Bacc("TRN2", target_bir_lowering=False, debug=False)
    bf16 = mybir.dt.bfloat16
    uu = nc.dram_tensor("uu", [NCH, 128, NGRP, 2, GCOL], bf16, kind="ExternalInput")
    out_d = nc.dram_tensor("loss_part", [1, 1], f32, kind="ExternalOutput")
    dbg = {}
    if debug_dump:
        for nm, shp in (
            ("d_xt", [128, 2, 24, 24]), ("d_yt", [128, 2, 12, 24]),
            ("d_at", [128, 2, 12, 24]),
            ("d_wd", [128, 2, 12]), ("d_con", [128, 2, 12]),
            ("d_invd", [128, 2, 12]),
        ):
            dbg[nm] = nc.dram_tensor(nm, shp, f32, kind="ExternalOutput")

    with tile.TileContext(nc) as tc:
        with (
            tc.tile_pool(name="slab", bufs=10) as slab_pool,
            tc.tile_pool(name="stage", bufs=3) as stage_pool,
            tc.tile_pool(name="solve", bufs=1) as solve_pool,
            tc.tile_pool(name="psum", bufs=7, space=bass.MemorySpace.PSUM) as psum_pool,
            tc.tile_pool(name="fpsum", bufs=1, space=bass.MemorySpace.PSUM) as fpsum_pool,
            tc.tile_pool(name="scr", bufs=1, space=bass.MemorySpace.DRAM) as scr_pool,
        ):
            # DRAM bounce buffer for gram extraction: [96 rows, 64 groups, 288]
            scr_t = scr_pool.tile([2, 128, 2, 24, 24], f32)
            # persistent solve-layout tiles (slot dim: 2 halves of 128 blocks)
            xt = solve_pool.tile([128, 2, 24, 24], f32)   # RE full; X-plane = rows 0:12
            yt = solve_pool.tile([128, 2, 12, 24], f32)   # Im plane [Im G | Im r]
            af = solve_pool.tile([128, 2, 24, 24], f32)   # A full
            invd = solve_pool.tile([128, 2, 12], f32)
            axr = solve_pool.tile([128, 2, 23], f32)
            bxr = solve_pool.tile([128, 2, 23], f32)
            t1 = solve_pool.tile([128, 2, 11, 23], f32)
            t2 = solve_pool.tile([128, 2, 11, 23], f32)
            t3 = solve_pool.tile([128, 2, 11, 23], f32)
            t4 = solve_pool.tile([128, 2, 11, 23], f32)
            wd = solve_pool.tile([128, 2, 12], f32)
            sq = solve_pool.tile([128, 2, 12, 12], f32)   # (v, k) layout
            sq2 = solve_pool.tile([128, 2, 12, 12], f32)
            sig = solve_pool.tile([128, 2, 12], f32)
            con = solve_pool.tile([128, 2, 12], f32)
            ones = solve_pool.tile([128, 1], f32)
            fsb = solve_pool.tile([1, 24], f32)
            fin = solve_pool.tile([1, 12], f32)
            fin2 = solve_pool.tile([1, 1], f32)

            nc.vector.memset(ones[:], 1.0)

            def solve_half(h):
                """Batched complex LDL^H for 128 blocks at slot h."""
                sl = slice(h, h + 1)
                # Y = A[0:12,:] - A^T[0:12,:] via transposed free view
                a_rows = af[:, sl, 0:12, 0:24]
                a_colsT = af[:, sl, 0:24, 0:12].transpose([0, 1, 3, 2])
                nc.vector.tensor_tensor(yt[:, sl], a_rows, a_colsT, op=Alu.subtract)
                # Wd[u] = RE[12+u, 12+u]
                xt_flat = xt[:].rearrange("p s r c -> p s (r c)")
                diag_ap = xt_flat[:, sl, 300:576:25]
                nc.vector.tensor_copy(wd[:, sl], diag_ap)
                if debug_dump and h == 1:
                    nc.sync.dma_start(dbg["d_xt"].ap(), xt[:])
                    nc.sync.dma_start(dbg["d_yt"].ap(), yt[:])
                    nc.sync.dma_start(dbg["d_at"].ap(), af[:, :, 0:12, :])
                    nc.sync.dma_start(dbg["d_wd"].ap(), wd[:])
                for k in range(12):
                    nr = 11 - k            # rows k+1..11
                    ncols = 23 - k         # cols k+1..23
                    nc.vector.reciprocal(invd[:, sl, k], xt[:, sl, k, k])
                    ib = invd[:, sl, k].unsqueeze(2).broadcast_to([128, 1, ncols])
                    nc.vector.tensor_tensor(
                        axr[:, sl, 0:ncols], xt[:, sl, k, k + 1:24], ib, op=Alu.mult)
                    nc.vector.tensor_tensor(
                        bxr[:, sl, 0:ncols], yt[:, sl, k, k + 1:24], ib, op=Alu.mult)
                    if nr == 0:
                        continue
                    shp = [128, 1, nr, ncols]
                    xc = xt[:, sl, k + 1:12, k].unsqueeze(3).broadcast_to(shp)
                    yc = yt[:, sl, k + 1:12, k].unsqueeze(3).broadcast_to(shp)
                    axb = axr[:, sl, 0:ncols].unsqueeze(2).broadcast_to(shp)
                    bxb = bxr[:, sl, 0:ncols].unsqueeze(2).broadcast_to(shp)
                    tt1 = t1[:, sl, 0:nr, 0:ncols]
                    tt2 = t2[:, sl, 0:nr, 0:ncols]
                    tt3 = t3[:, sl, 0:nr, 0:ncols]
                    tt4 = t4[:, sl, 0:nr, 0:ncols]
                    nc.vector.tensor_tensor(tt1, xc, axb, op=Alu.mult)
                    nc.vector.tensor_tensor(tt2, yc, bxb, op=Alu.mult)
                    nc.vector.tensor_tensor(tt3, yc, axb, op=Alu.mult)
                    nc.gpsimd.tensor_tensor(tt4, xc, bxb, op=Alu.mult)
                    xrect = xt[:, sl, k + 1:12, k + 1:24]
                    yrect = yt[:, sl, k + 1:12, k + 1:24]
                    nc.vector.tensor_tensor(xrect, xrect, tt1, op=Alu.subtract)
                    nc.vector.tensor_tensor(xrect, xrect, tt2, op=Alu.add)
                    nc.gpsimd.tensor_tensor(yrect, yrect, tt3, op=Alu.subtract)
                    nc.gpsimd.tensor_tensor(yrect, yrect, tt4, op=Alu.subtract)
                # sigma_v = sum_k (zr^2 + zi^2) * invd_k   in (v, k) layout
                zr = xt[:, sl, 0:12, 12:24].transpose([0, 1, 3, 2])
                zi = yt[:, sl, 0:12, 12:24].transpose([0, 1, 3, 2])
                nc.vector.tensor_tensor(sq[:, sl], zr, zr, op=Alu.mult)
                nc.vector.tensor_tensor(sq2[:, sl], zi, zi, op=Alu.mult)
                nc.vector.tensor_tensor(sq[:, sl], sq[:, sl], sq2[:, sl], op=Alu.add)
                idb = invd[:, sl].unsqueeze(2).broadcast_to([128, 1, 12, 12])
                nc.vector.tensor_tensor(sq[:, sl], sq[:, sl], idb, op=Alu.mult)
                nc.vector.tensor_reduce(
                    sig[:, sl].unsqueeze(3), sq[:, sl],
                    axis=mybir.AxisListType.X, op=Alu.add)
                nc.vector.tensor_tensor(con[:, sl], wd[:, sl], sig[:, sl], op=Alu.subtract)
                if debug_dump and h == 1:
                    nc.sync.dma_start(dbg["d_con"].ap(), con[:])
                    nc.sync.dma_start(dbg["d_invd"].ap(), invd[:])

            # ---------------- streaming phase: grams on PE ----------------
            QG = NGRP // 4  # 16 groups per quarter
            scr_ap0 = scr_t[:]
            scr_h = scr_ap0.tensor  # handle for raw AP construction
            scr_base = scr_ap0.offset
            for q in range(4):  # quarters of the group range
                slabs = []
                for c in range(NCH):
                    st = slab_pool.tile([128, QG, 2, GCOL], bf16, tag="slab")
                    nc.sync.dma_start(
                        st[:], uu.ap()[c, :, q * QG:(q + 1) * QG, :, :]
                    )
                    slabs.append(st)
                stage_t = stage_pool.tile([96, QG, 192], f32, tag="stage")
                for gl in range(QG):
                    ps = psum_pool.tile([96, 192], f32)
                    for c in range(NCH):
                        ure = slabs[c][:, gl, 0, :]
                        uim = slabs[c][:, gl, 1, :]
                        full = slabs[c][:, gl, :, :]
                        first = c == 0
                        last = c == NCH - 1
                        # [RE | A] += Ure^T [Ure | Uim]
                        nc.tensor.matmul(
                            ps[:, 0:192], ure, full,
                            start=first, stop=False, skip_group_check=True,
                        )
                        # RE += Uim^T Uim
                        nc.tensor.matmul(
                            ps[:, 0:96], uim, uim,
                            start=False, stop=last, skip_group_check=True,
                        )
                    nc.scalar.copy(stage_t[:, gl, :], ps[:])
                # bounce only per-block diag pieces, directly in solve layout:
                # scr2[h][P=32j+16*(q%2)+g][plane][i][c]
                h_q = q // 2
                for j in range(GRP):
                    for plane, colbase in ((0, 0), (1, 96)):
                        src = stage_t[24 * j:24 * j + 24, :,
                                      colbase + 24 * j:colbase + 24 * j + 24]
                        off = (scr_base + h_q * 147456
                               + (32 * j + 16 * (q % 2)) * 1152 + plane * 576)
                        dst = bass.AP(scr_h, off, [[24, 24], [1152, QG], [1, 24]])
                        nc.scalar.dma_start(dst, src)

                if q % 2 == 1:
                    # gather DRAM scratch -> blocks-in-partitions solve layout.
                    # block (g_loc, j) of half -> partition 4*g_loc + j, slot h.
                    # scr flat strides: row 18432, group 288, col 1.
                    h = q // 2
                    base = scr_base + h * 147456
                    src_re = bass.AP(scr_h, base, [[1152, 128], [24, 24], [1, 24]])
                    nc.sync.dma_start(xt[:, h, :, :], src_re)
                    src_a = bass.AP(scr_h, base + 576, [[1152, 128], [24, 24], [1, 24]])
                    nc.sync.dma_start(af[:, h, :, :], src_a)
                    solve_half(h)

            # ---------------- final reduction ----------------
            fps = fpsum_pool.tile([1, 24], f32)
            nc.tensor.matmul(fps[:], ones[:], con[:], start=True, stop=True)
            nc.vector.tensor_copy(fsb[:], fps[:])
            nc.vector.tensor_tensor(
                fin[:], fsb[:, 0:12], fsb[:, 12:24], op=Alu.add)
            nc.scalar.activation(fin[:], fin[:], Act.Sqrt)
            nc.vector.tensor_reduce(
                fin2[:].unsqueeze(2), fin[:].unsqueeze(1),
                axis=mybir.AxisListType.X, op=Alu.add)
            nc.vector.tensor_scalar(
                fin2[:], fin2[:], 1.0 / (N_CORES * NV), None, op0=Alu.mult)
            nc.scalar.dma_start(out_d.ap(), fin2[:])

    nc.compile()
    _PROGRAM_CACHE[key] = (nc, uu.name, out_d.name)
    return _PROGRAM_CACHE[key]


def kernel(pred_re, pred_im, target_re, target_im):
    pred_re = np.ascontiguousarray(pred_re, np.float32)
    pred_im = np.ascontiguousarray(pred_im, np.float32)
    target_re = np.ascontiguousarray(target_re, np.float32)
    target_im = np.ascontiguousarray(target_im, np.float32)

    uus = _host_layout(pred_re, pred_im, target_re, target_im)
    nc, in_name, out_name = build_program()

    from concourse.bass_utils import run_bass_kernel_spmd

    in_maps = [{in_name: uus[b]} for b in range(N_CORES)]
    res = run_bass_kernel_spmd(nc, in_maps, core_ids=list(range(N_CORES)))
    total = np.float32(0.0)
    for r in res.results:
        total += r[out_name].reshape(-1)[0]
    return np.float32(total)


if __name__ == "__main__":
    import jax
    import reference

    cpu = jax.devices("cpu")[0]
    with jax.default_device(cpu):
        inputs = {k: np.asarray(jax.device_put(v, cpu)) for k, v in reference.setup_inputs().items()}
        expected = np.asarray(reference.reference(**inputs))
    got = kernel(**inputs)
    rel = abs(got - expected) / abs(expected)
    print(f"expected={expected} got={got} rel={rel:.3e}")
